# revision 4
# baseline (speedup 1.0000x reference)
"""TRN2 Bass kernel for nn_LinearBinary: out = (A @ W + b) +/- 1 per-row.

    A: [8192, 2048] f32, W: [2048, 2048] f32, b: [2048] f32
    C = A @ W + b;  cond = C[:, :1] > 0.5;  out = where(cond, C+1, C-1)

Sharding: data-parallel over the 8192-row batch across 8 NeuronCores
(1024 rows/core); W and b replicated. SPMD - one program, per-core shards
via in_maps.

Per-core kernel (v2 — PE-minimal):
  - W shipped as bf16 from the host (halves W DMA to 8MB) and kept fully
    resident in SBUF as 4 ko-groups [kp=128, 4, 2048], streamed on the
    sync (SP) HWDGE queue.
  - A^T shipped pre-transposed bf16 from the host ([K, M] layout) so the
    PE does ZERO transposes — only the 512 bf16 matmuls (1 cyc/row,
    262144 cycles ~= 109us at 2.4GHz). Loaded per m-tile-pair on the
    scalar (Activation) HWDGE queue.
  - m-tiles processed in PAIRS with ko-group-outer order across all 8
    PSUM banks, so the first pair's matmuls start after just the first
    2MB W group + 1MB A^T land, and later W groups stream in behind the
    PE.
  - The row condition needs exact fp32 C[:, 0] (min |C0-0.5| margin on
    this data is ~4.4e-4; bf16 A would flip rows): computed on
    gpsimd (mult) + DVE (reduce) from the natural-layout fp32 A, exactly
    as in v1.
  - Epilogue fuses (psum + (-+1)) + b in one scalar_tensor_tensor per
    [128, 512] tile; stores go back on the sync queue (idle once W has
    landed).
"""

import sys

for _p in ("/opt/trn_rl_repo", "/root/.axon_site/_ro/trn_rl_repo"):
    if _p not in sys.path:
        sys.path.append(_p)

import ml_dtypes
import numpy as np

import concourse.bacc as bacc
import concourse.mybir as mybir
import concourse.tile as tile
from concourse.bass_utils import run_bass_kernel_spmd

dt = mybir.dt
Alu = mybir.AluOpType
BF16 = np.dtype(ml_dtypes.bfloat16)

P = 128
K = 2048
N = 2048
B_FULL = 8192
N_CORES = 8
M_SHARD = B_FULL // N_CORES  # 1024 rows per core
M_TILES = M_SHARD // P  # 8
KO = K // P  # 16
KG = 4  # W ko-groups (DMA granularity)
KGS = KO // KG  # 4 ko per group
NQ = 4  # PSUM n-chunks
N_SUB = N // NQ  # 512
PAIRS = M_TILES // 2  # m-tiles processed 2 at a time (8 PSUM banks)


def _build():
    nc = bacc.Bacc("TRN2", target_bir_lowering=False, debug=False, num_devices=N_CORES)

    a = nc.dram_tensor("inputs", [M_SHARD, K], dt.float32, kind="ExternalInput")
    at = nc.dram_tensor("at", [K, M_SHARD], dt.bfloat16, kind="ExternalInput")
    w = nc.dram_tensor("w", [K, N], dt.bfloat16, kind="ExternalInput")
    b = nc.dram_tensor("b", [N], dt.float32, kind="ExternalInput")
    # W[:, 0] pre-sliced on host: a strided 4-byte column-gather DMA is fatal
    # on HW (NRT_EXEC_UNIT_UNRECOVERABLE), so ship the 8KB row directly.
    w0 = nc.dram_tensor("w0", [1, K], dt.float32, kind="ExternalInput")
    out = nc.dram_tensor("out", [M_SHARD, N], dt.float32, kind="ExternalOutput")

    # [kp, ko, *] views (contraction partitions first)
    w_kpn = w.ap().rearrange("(ko kp) n -> kp ko n", kp=P)
    at_kpm = at.ap().rearrange("(ko kp) m -> kp ko m", kp=P)

    with tile.TileContext(nc) as tc:
        with (
            tc.tile_pool(name="consts", bufs=1) as consts,
            tc.tile_pool(name="wg", bufs=1) as wg_pool,
            tc.tile_pool(name="atp", bufs=1) as at_pool,
            tc.tile_pool(name="anat", bufs=2) as anat_pool,
            tc.tile_pool(name="scr", bufs=2) as scr_pool,
            tc.tile_pool(name="dsm", bufs=1) as d_pool,
            tc.tile_pool(name="outs", bufs=4) as out_pool,
            tc.tile_pool(name="psc", bufs=8, space="PSUM") as psum_pool,
        ):
            # b and W[:, 0] broadcast to all partitions (scalar queue: tiny)
            b_row = consts.tile([1, N], dt.float32, tag="b_row")
            nc.scalar.dma_start(b_row[:], b.ap().unsqueeze(0))
            b128 = consts.tile([P, N], dt.float32, tag="b128")
            nc.gpsimd.partition_broadcast(b128[:], b_row[:])
            w0_row = consts.tile([1, K], dt.float32, tag="w0_row")
            nc.scalar.dma_start(w0_row[:], w0.ap())
            w0b = consts.tile([P, K], dt.float32, tag="w0b")
            nc.gpsimd.partition_broadcast(w0b[:], w0_row[:])

            # W resident in SBUF, bf16, 4 ko-groups on the sync queue.
            wgs = []
            for g in range(KG):
                wg = wg_pool.tile([P, KGS, N], dt.bfloat16, tag=f"wg{g}")
                nc.sync.dma_start(wg[:], w_kpn[:, g * KGS : (g + 1) * KGS, :])
                wgs.append(wg)

            for pr in range(PAIRS):
                m0 = 2 * pr
                # A^T for this pair of m-tiles (bf16, scalar queue)
                atp = at_pool.tile([P, KO, 2 * P], dt.bfloat16, tag=f"atp{pr}")
                nc.scalar.dma_start(atp[:], at_kpm[:, :, m0 * P : (m0 + 2) * P])

                # exact fp32 condition for both m-tiles
                ds = []
                for m in (m0, m0 + 1):
                    a_nat = anat_pool.tile([P, K], dt.float32, tag="a_nat")
                    nc.scalar.dma_start(a_nat[:], a.ap()[m * P : (m + 1) * P, :])
                    scratch = scr_pool.tile([P, K], dt.float32, tag="scratch")
                    c0 = d_pool.tile([P, 1], dt.float32, tag=f"c0_{m}")
                    nc.gpsimd.tensor_tensor(scratch[:], a_nat[:], w0b[:], Alu.mult)
                    nc.vector.tensor_reduce(
                        c0[:], scratch[:], mybir.AxisListType.X, Alu.add
                    )
                    g = d_pool.tile([P, 1], dt.float32, tag=f"g_{m}")
                    nc.vector.tensor_scalar(
                        g[:], c0[:], b128[:, 0:1], 0.5, Alu.add, Alu.is_gt
                    )
                    d = d_pool.tile([P, 1], dt.float32, tag=f"d_{m}")
                    nc.vector.tensor_scalar(d[:], g[:], 2.0, -1.0, Alu.mult, Alu.add)
                    ds.append(d)

                # matmuls: ko-group outer so the pair streams behind W DMAs,
                # 8 accumulation groups interleaved across the 8 PSUM banks.
                psums = [
                    [
                        psum_pool.tile(
                            [P, N_SUB], dt.float32, name=f"ps_{mi}_{nq}", tag="psum"
                        )
                        for nq in range(NQ)
                    ]
                    for mi in range(2)
                ]
                for g4 in range(KG):
                    for kk in range(KGS):
                        ko = g4 * KGS + kk
                        for mi in range(2):
                            for nq in range(NQ):
                                nc.tensor.matmul(
                                    psums[mi][nq][:],
                                    atp[:, ko, mi * P : (mi + 1) * P],
                                    wgs[g4][:, kk, nq * N_SUB : (nq + 1) * N_SUB],
                                    start=(ko == 0),
                                    stop=(ko == KO - 1),
                                )

                # epilogue: out = (psum + d) + b, store on sync queue
                for mi in range(2):
                    m = m0 + mi
                    for nq in range(NQ):
                        out_sb = out_pool.tile([P, N_SUB], dt.float32, tag="out_sb")
                        nc.vector.scalar_tensor_tensor(
                            out_sb[:],
                            psums[mi][nq][:],
                            ds[mi][:],
                            b128[:, nq * N_SUB : (nq + 1) * N_SUB],
                            Alu.add,
                            Alu.add,
                        )
                        nc.sync.dma_start(
                            out.ap()[m * P : (m + 1) * P, nq * N_SUB : (nq + 1) * N_SUB],
                            out_sb[:],
                        )

    nc.compile()
    return nc


_NC = None


def _get_nc():
    global _NC
    if _NC is None:
        _NC = _build()
    return _NC


def build_in_maps(a, w, b):
    """Host-side prep: shard A, pre-transpose to bf16, cast W to bf16."""
    a = np.ascontiguousarray(a, dtype=np.float32)
    w = np.ascontiguousarray(w, dtype=np.float32)
    b = np.ascontiguousarray(b, dtype=np.float32)
    w_bf = w.astype(BF16)
    w0 = np.ascontiguousarray(w[:, 0].reshape(1, K))
    in_maps = []
    for i in range(N_CORES):
        a_sh = a[i * M_SHARD : (i + 1) * M_SHARD]
        in_maps.append(
            {
                "inputs": a_sh,
                "at": a_sh.T.astype(BF16),  # [K, M_SHARD] bf16, C-contiguous
                "w": w_bf,
                "b": b,
                "w0": w0,
            }
        )
    return in_maps


def kernel(**inputs: np.ndarray) -> np.ndarray:
    a = inputs["inputs"]
    assert a.shape == (B_FULL, K), a.shape
    nc = _get_nc()
    in_maps = build_in_maps(a, inputs["w"], inputs["b"])
    res = run_bass_kernel_spmd(nc, in_maps, core_ids=list(range(N_CORES)))
    return np.concatenate([res.results[i]["out"] for i in range(N_CORES)], axis=0)


# revision 5
# speedup vs baseline: 1.0939x; 1.0939x over previous
"""TRN2 Bass kernel for nn_LinearBinary: out = (A @ W + b) +/- 1 per-row.

    A: [8192, 2048] f32, W: [2048, 2048] f32, b: [2048] f32
    C = A @ W + b;  cond = C[:, :1] > 0.5;  out = where(cond, C+1, C-1)

Sharding: data-parallel over the 8192-row batch across 8 NeuronCores
(1024 rows/core); W and b replicated. SPMD - one program, per-core shards
via in_maps.

Per-core kernel (v3 — PE-minimal, DMA-tuned):
  - W shipped as bf16 (8MB) and kept fully resident in SBUF as 4
    ko-groups, streamed on the sync (SP) HWDGE queue with 4KB packets.
  - A^T shipped pre-transposed bf16 from the host, blocked per m-pair as
    [pair, kp, ko, m] so each partition line is one contiguous 8KB run:
    DMA arbitration is packet-round-robin, so packet size IS bandwidth
    share — 512B A^T packets get starved 8:1 by 4KB W packets.
  - PE does ZERO transposes — only the 512 bf16 matmuls (1 cyc/row,
    262144 cycles ~= 109us at 2.4GHz).
  - m-tiles in PAIRS across all 8 PSUM banks. Pair 0 runs ko-group-outer
    so its matmuls stream right behind the W group DMAs; pairs 1-3 run
    ko-inner per psum so each [128,512] tile closes early and its
    epilogue+store overlaps the remaining matmuls (kills the end-of-pair
    store burst and the final tail).
  - The row condition needs exact fp32 C[:, 0] (min |C0-0.5| margin on
    this data is ~4.4e-4; bf16 A would flip rows): computed on
    gpsimd (mult) + DVE (reduce) from the natural-layout fp32 A.
  - Epilogue fuses (psum + (-+1)) + b in one scalar_tensor_tensor per
    [128, 512] tile; stores go on the sync queue (idle once W landed).
"""

import sys

for _p in ("/opt/trn_rl_repo", "/root/.axon_site/_ro/trn_rl_repo"):
    if _p not in sys.path:
        sys.path.append(_p)

import ml_dtypes
import numpy as np

import concourse.bacc as bacc
import concourse.mybir as mybir
import concourse.tile as tile
from concourse.bass_utils import run_bass_kernel_spmd

dt = mybir.dt
Alu = mybir.AluOpType
BF16 = np.dtype(ml_dtypes.bfloat16)

P = 128
K = 2048
N = 2048
B_FULL = 8192
N_CORES = 8
M_SHARD = B_FULL // N_CORES  # 1024 rows per core
M_TILES = M_SHARD // P  # 8
KO = K // P  # 16
KG = 4  # W ko-groups (DMA granularity)
KGS = KO // KG  # 4 ko per group
NQ = 4  # PSUM n-chunks
N_SUB = N // NQ  # 512
PAIRS = M_TILES // 2  # m-tiles processed 2 at a time (8 PSUM banks)
MP = 2 * P  # rows per pair


def _build():
    nc = bacc.Bacc("TRN2", target_bir_lowering=False, debug=False, num_devices=N_CORES)

    a = nc.dram_tensor("inputs", [M_SHARD, K], dt.float32, kind="ExternalInput")
    # A^T, host-blocked per m-pair: at[pr, kp, ko, mm] = A[pr*256+mm, ko*128+kp]
    at = nc.dram_tensor("at", [PAIRS, P, KO, MP], dt.bfloat16, kind="ExternalInput")
    w = nc.dram_tensor("w", [K, N], dt.bfloat16, kind="ExternalInput")
    b = nc.dram_tensor("b", [N], dt.float32, kind="ExternalInput")
    # W[:, 0] pre-sliced on host: a strided 4-byte column-gather DMA is fatal
    # on HW (NRT_EXEC_UNIT_UNRECOVERABLE), so ship the 8KB row directly.
    w0 = nc.dram_tensor("w0", [1, K], dt.float32, kind="ExternalInput")
    out = nc.dram_tensor("out", [M_SHARD, N], dt.float32, kind="ExternalOutput")

    w_kpn = w.ap().rearrange("(ko kp) n -> kp ko n", kp=P)

    with tile.TileContext(nc) as tc:
        with (
            tc.tile_pool(name="consts", bufs=1) as consts,
            tc.tile_pool(name="wg", bufs=1) as wg_pool,
            tc.tile_pool(name="atp", bufs=1) as at_pool,
            tc.tile_pool(name="anat", bufs=2) as anat_pool,
            tc.tile_pool(name="scr", bufs=2) as scr_pool,
            tc.tile_pool(name="dsm", bufs=1) as d_pool,
            tc.tile_pool(name="outs", bufs=4) as out_pool,
            tc.tile_pool(name="psc", bufs=8, space="PSUM") as psum_pool,
        ):
            # b and W[:, 0] broadcast to all partitions (scalar queue: tiny)
            b_row = consts.tile([1, N], dt.float32, tag="b_row")
            nc.scalar.dma_start(b_row[:], b.ap().unsqueeze(0))
            b128 = consts.tile([P, N], dt.float32, tag="b128")
            nc.gpsimd.partition_broadcast(b128[:], b_row[:])
            w0_row = consts.tile([1, K], dt.float32, tag="w0_row")
            nc.scalar.dma_start(w0_row[:], w0.ap())
            w0b = consts.tile([P, K], dt.float32, tag="w0b")
            nc.gpsimd.partition_broadcast(w0b[:], w0_row[:])

            # W resident, bf16, 4 ko-groups on the sync queue
            wgs = []
            for g in range(KG):
                wg = wg_pool.tile([P, KGS, N], dt.bfloat16, tag=f"wg{g}")
                nc.sync.dma_start(wg[:], w_kpn[:, g * KGS : (g + 1) * KGS, :])
                wgs.append(wg)

            # all A^T pair tiles up-front on the scalar queue (8KB packets),
            # ahead of the a_nat loads
            atps = []
            for pr in range(PAIRS):
                atp = at_pool.tile([P, KO, MP], dt.bfloat16, tag=f"atp{pr}", name=f"atp{pr}")
                nc.scalar.dma_start(atp[:], at.ap()[pr])
                atps.append(atp)

            def cond_m(m):
                # exact fp32 condition: d = (A[m-tile] @ w0 + b0 > 0.5) ? +1 : -1
                a_nat = anat_pool.tile([P, K], dt.float32, tag="a_nat", name="a_nat")
                nc.scalar.dma_start(a_nat[:], a.ap()[m * P : (m + 1) * P, :])
                scratch = scr_pool.tile([P, K], dt.float32, tag="scratch", name="scratch")
                c0 = d_pool.tile([P, 1], dt.float32, tag=f"c0_{m}", name=f"c0_{m}")
                nc.gpsimd.tensor_tensor(scratch[:], a_nat[:], w0b[:], Alu.mult)
                nc.vector.tensor_reduce(c0[:], scratch[:], mybir.AxisListType.X, Alu.add)
                g = d_pool.tile([P, 1], dt.float32, tag=f"g_{m}", name=f"g_{m}")
                nc.vector.tensor_scalar(
                    g[:], c0[:], b128[:, 0:1], 0.5, Alu.add, Alu.is_gt
                )
                d = d_pool.tile([P, 1], dt.float32, tag=f"d_{m}", name=f"d_{m}")
                nc.vector.tensor_scalar(d[:], g[:], 2.0, -1.0, Alu.mult, Alu.add)
                return d

            def epilogue(psum, d, m, nq):
                out_sb = out_pool.tile([P, N_SUB], dt.float32, tag="out_sb", name="out_sb")
                nc.vector.scalar_tensor_tensor(
                    out_sb[:],
                    psum[:],
                    d[:],
                    b128[:, nq * N_SUB : (nq + 1) * N_SUB],
                    Alu.add,
                    Alu.add,
                )
                nc.sync.dma_start(
                    out.ap()[m * P : (m + 1) * P, nq * N_SUB : (nq + 1) * N_SUB],
                    out_sb[:],
                )

            for pr in range(PAIRS):
                m0 = 2 * pr
                ds = [cond_m(m0), cond_m(m0 + 1)]
                atp = atps[pr]

                if pr == 0:
                    # ko-group outer: stream right behind the W group DMAs
                    psums = [
                        [
                            psum_pool.tile(
                                [P, N_SUB], dt.float32, name=f"ps_{mi}_{nq}", tag="psum"
                            )
                            for nq in range(NQ)
                        ]
                        for mi in range(2)
                    ]
                    for g4 in range(KG):
                        for kk in range(KGS):
                            ko = g4 * KGS + kk
                            for mi in range(2):
                                for nq in range(NQ):
                                    nc.tensor.matmul(
                                        psums[mi][nq][:],
                                        atp[:, ko, mi * P : (mi + 1) * P],
                                        wgs[g4][:, kk, nq * N_SUB : (nq + 1) * N_SUB],
                                        start=(ko == 0),
                                        stop=(ko == KO - 1),
                                    )
                    for mi in range(2):
                        for nq in range(NQ):
                            epilogue(psums[mi][nq], ds[mi], m0 + mi, nq)
                else:
                    # W fully resident: ko-inner per psum so each tile closes
                    # early and its epilogue+store overlaps remaining matmuls
                    for mi in range(2):
                        for nq in range(NQ):
                            psum = psum_pool.tile(
                                [P, N_SUB], dt.float32, name="ps", tag="psum"
                            )
                            for ko in range(KO):
                                nc.tensor.matmul(
                                    psum[:],
                                    atp[:, ko, mi * P : (mi + 1) * P],
                                    wgs[ko // KGS][:, ko % KGS, nq * N_SUB : (nq + 1) * N_SUB],
                                    start=(ko == 0),
                                    stop=(ko == KO - 1),
                                )
                            epilogue(psum, ds[mi], m0 + mi, nq)

    nc.compile()
    return nc


_NC = None


def _get_nc():
    global _NC
    if _NC is None:
        _NC = _build()
    return _NC


def build_in_maps(a, w, b):
    """Host-side prep: shard A, pre-transpose/block to bf16, cast W to bf16."""
    a = np.ascontiguousarray(a, dtype=np.float32)
    w = np.ascontiguousarray(w, dtype=np.float32)
    b = np.ascontiguousarray(b, dtype=np.float32)
    w_bf = w.astype(BF16)
    w0 = np.ascontiguousarray(w[:, 0].reshape(1, K))
    in_maps = []
    for i in range(N_CORES):
        a_sh = a[i * M_SHARD : (i + 1) * M_SHARD]
        # at[pr, kp, ko, mm] = a_sh[pr*256+mm, ko*128+kp]
        at = np.transpose(
            a_sh.reshape(PAIRS, MP, KO, P), (0, 3, 2, 1)
        ).astype(BF16)
        in_maps.append(
            {"inputs": a_sh, "at": at, "w": w_bf, "b": b, "w0": w0}
        )
    return in_maps


def kernel(**inputs: np.ndarray) -> np.ndarray:
    a = inputs["inputs"]
    assert a.shape == (B_FULL, K), a.shape
    nc = _get_nc()
    in_maps = build_in_maps(a, inputs["w"], inputs["b"])
    res = run_bass_kernel_spmd(nc, in_maps, core_ids=list(range(N_CORES)))
    return np.concatenate([res.results[i]["out"] for i in range(N_CORES)], axis=0)


# revision 12
# speedup vs baseline: 1.1481x; 1.0496x over previous
"""TRN2 Bass kernel for nn_LinearBinary: out = (A @ W + b) +/- 1 per-row.

    A: [8192, 2048] f32, W: [2048, 2048] f32, b: [2048] f32
    C = A @ W + b;  cond = C[:, :1] > 0.5;  out = where(cond, C+1, C-1)

Sharding: data-parallel over the 8192-row batch across 8 NeuronCores
(1024 rows/core); W and b replicated. SPMD - one program, per-core shards
via in_maps.

Per-core kernel (v3 — PE-minimal, DMA-tuned):
  - W shipped as bf16 (8MB) and kept fully resident in SBUF as 4
    ko-groups, streamed on the sync (SP) HWDGE queue with 4KB packets.
  - A^T shipped pre-transposed bf16 from the host, blocked per m-pair as
    [pair, kp, ko, m] so each partition line is one contiguous 8KB run:
    DMA arbitration is packet-round-robin, so packet size IS bandwidth
    share — 512B A^T packets get starved 8:1 by 4KB W packets.
  - PE does ZERO transposes — only the 512 bf16 matmuls (1 cyc/row,
    262144 cycles ~= 109us at 2.4GHz).
  - m-tiles in PAIRS across all 8 PSUM banks. Pair 0 runs ko-group-outer
    so its matmuls stream right behind the W group DMAs; pairs 1-3 run
    ko-inner per psum so each [128,512] tile closes early and its
    epilogue+store overlaps the remaining matmuls (kills the end-of-pair
    store burst and the final tail).
  - The row condition needs exact fp32 C[:, 0] (min |C0-0.5| margin on
    this data is ~4.4e-4; bf16 A would flip rows): computed on
    gpsimd (mult) + DVE (reduce) from the natural-layout fp32 A.
  - Epilogue fuses (psum + (-+1)) + b in one scalar_tensor_tensor per
    [128, 512] tile; stores go on the sync queue (idle once W landed).
"""

import sys

for _p in ("/opt/trn_rl_repo", "/root/.axon_site/_ro/trn_rl_repo"):
    if _p not in sys.path:
        sys.path.append(_p)

import ml_dtypes
import numpy as np

import concourse.bacc as bacc
import concourse.mybir as mybir
import concourse.tile as tile
from concourse.bass_utils import run_bass_kernel_spmd

dt = mybir.dt
Alu = mybir.AluOpType
BF16 = np.dtype(ml_dtypes.bfloat16)

P = 128
K = 2048
N = 2048
B_FULL = 8192
N_CORES = 8
M_SHARD = B_FULL // N_CORES  # 1024 rows per core
M_TILES = M_SHARD // P  # 8
KO = K // P  # 16
KG = 8  # W ko-groups (DMA granularity)
KGS = KO // KG  # 2 ko per group
NQ = 4  # PSUM n-chunks
N_SUB = N // NQ  # 512
PAIRS = M_TILES // 2  # m-tiles processed 2 at a time (8 PSUM banks)
MP = 2 * P  # rows per pair


def _build():
    nc = bacc.Bacc("TRN2", target_bir_lowering=False, debug=False, num_devices=N_CORES)

    a = nc.dram_tensor("inputs", [M_SHARD, K], dt.float32, kind="ExternalInput")
    # A^T, host-blocked per m-pair: at[pr, kp, ko, mm] = A[pr*256+mm, ko*128+kp]
    at = nc.dram_tensor("at", [PAIRS, P, KO, MP], dt.bfloat16, kind="ExternalInput")
    # W host-blocked per ko-group: w[g, kp, kk, n] = W[(g*KGS+kk)*128+kp, n]
    # so each partition line is one contiguous 8KB run (big DMA packets win
    # the packet-round-robin arbitration).
    w = nc.dram_tensor("w", [KG, P, KGS, N], dt.bfloat16, kind="ExternalInput")
    b = nc.dram_tensor("b", [N], dt.float32, kind="ExternalInput")
    # W[:, 0] pre-sliced on host: a strided 4-byte column-gather DMA is fatal
    # on HW (NRT_EXEC_UNIT_UNRECOVERABLE), so ship the 8KB row directly.
    w0 = nc.dram_tensor("w0", [1, K], dt.float32, kind="ExternalInput")
    out = nc.dram_tensor("out", [M_SHARD, N], dt.float32, kind="ExternalOutput")

    with tile.TileContext(nc) as tc:
        with (
            tc.tile_pool(name="consts", bufs=1) as consts,
            tc.tile_pool(name="wg", bufs=1) as wg_pool,
            tc.tile_pool(name="atp", bufs=1) as at_pool,
            tc.tile_pool(name="anat", bufs=2) as anat_pool,
            tc.tile_pool(name="scr", bufs=2) as scr_pool,
            tc.tile_pool(name="dsm", bufs=1) as d_pool,
            tc.tile_pool(name="outs", bufs=4) as out_pool,
            tc.tile_pool(name="psc", bufs=8, space="PSUM") as psum_pool,
        ):
            # b and W[:, 0] broadcast to all partitions (scalar queue: tiny)
            b_row = consts.tile([1, N], dt.float32, tag="b_row")
            nc.scalar.dma_start(b_row[:], b.ap().unsqueeze(0))
            b128 = consts.tile([P, N], dt.float32, tag="b128")
            nc.gpsimd.partition_broadcast(b128[:], b_row[:])
            w0_row = consts.tile([1, K], dt.float32, tag="w0_row")
            nc.scalar.dma_start(w0_row[:], w0.ap())
            w0b = consts.tile([P, K], dt.float32, tag="w0b")
            nc.gpsimd.partition_broadcast(w0b[:], w0_row[:])

            # W resident, bf16, 8 ko-groups of 1MB on the sync queue
            wgs = []
            for g in range(KG):
                wg = wg_pool.tile([P, KGS, N], dt.bfloat16, tag=f"wg{g}")
                nc.sync.dma_start(wg[:], w.ap()[g])
                wgs.append(wg)

            # A^T pair tiles on the scalar queue (8KB packets): atp0/atp1
            # first (critical for the PE start), the rest interleaved with
            # the a_nat condition loads below.
            atps = []
            for pr in range(PAIRS):
                atp = at_pool.tile([P, KO, MP], dt.bfloat16, tag=f"atp{pr}", name=f"atp{pr}")
                atps.append(atp)

            def load_atp(pr):
                nc.scalar.dma_start(atps[pr][:], at.ap()[pr])

            load_atp(0)
            load_atp(1)

            def cond_m(m):
                # exact fp32 condition: d = (A[m-tile] @ w0 + b0 > 0.5) ? +1 : -1
                a_nat = anat_pool.tile([P, K], dt.float32, tag="a_nat", name="a_nat")
                nc.scalar.dma_start(a_nat[:], a.ap()[m * P : (m + 1) * P, :])
                scratch = scr_pool.tile([P, K], dt.float32, tag="scratch", name="scratch")
                c0 = d_pool.tile([P, 1], dt.float32, tag=f"c0_{m}", name=f"c0_{m}")
                nc.vector.tensor_tensor(scratch[:], a_nat[:], w0b[:], Alu.mult)
                nc.vector.tensor_reduce(c0[:], scratch[:], mybir.AxisListType.X, Alu.add)
                g = d_pool.tile([P, 1], dt.float32, tag=f"g_{m}", name=f"g_{m}")
                nc.vector.tensor_scalar(
                    g[:], c0[:], b128[:, 0:1], 0.5, Alu.add, Alu.is_gt
                )
                d = d_pool.tile([P, 1], dt.float32, tag=f"d_{m}", name=f"d_{m}")
                nc.vector.tensor_scalar(d[:], g[:], 2.0, -1.0, Alu.mult, Alu.add)
                return d

            def epilogue(psum, d, m, nq):
                out_sb = out_pool.tile([P, N_SUB], dt.float32, tag="out_sb", name="out_sb")
                nc.vector.scalar_tensor_tensor(
                    out_sb[:],
                    psum[:],
                    d[:],
                    b128[:, nq * N_SUB : (nq + 1) * N_SUB],
                    Alu.add,
                    Alu.add,
                )
                nc.sync.dma_start(
                    out.ap()[m * P : (m + 1) * P, nq * N_SUB : (nq + 1) * N_SUB],
                    out_sb[:],
                )

            for pr in range(PAIRS):
                m0 = 2 * pr
                ds = [cond_m(m0), cond_m(m0 + 1)]
                if pr == 0:
                    load_atp(2)
                    load_atp(3)
                atp = atps[pr]

                if pr == 0:
                    # ko-group outer: stream right behind the W group DMAs
                    psums = [
                        [
                            psum_pool.tile(
                                [P, N_SUB], dt.float32, name=f"ps_{mi}_{nq}", tag="psum"
                            )
                            for nq in range(NQ)
                        ]
                        for mi in range(2)
                    ]
                    for g4 in range(KG):
                        for kk in range(KGS):
                            ko = g4 * KGS + kk
                            for mi in range(2):
                                for nq in range(NQ):
                                    nc.tensor.matmul(
                                        psums[mi][nq][:],
                                        atp[:, ko, mi * P : (mi + 1) * P],
                                        wgs[g4][:, kk, nq * N_SUB : (nq + 1) * N_SUB],
                                        start=(ko == 0),
                                        stop=(ko == KO - 1),
                                    )
                    for mi in range(2):
                        for nq in range(NQ):
                            epilogue(psums[mi][nq], ds[mi], m0 + mi, nq)
                else:
                    # W fully resident: ko-inner per psum so each tile closes
                    # early and its epilogue+store overlaps remaining matmuls
                    for mi in range(2):
                        for nq in range(NQ):
                            psum = psum_pool.tile(
                                [P, N_SUB], dt.float32, name="ps", tag="psum"
                            )
                            for ko in range(KO):
                                nc.tensor.matmul(
                                    psum[:],
                                    atp[:, ko, mi * P : (mi + 1) * P],
                                    wgs[ko // KGS][:, ko % KGS, nq * N_SUB : (nq + 1) * N_SUB],
                                    start=(ko == 0),
                                    stop=(ko == KO - 1),
                                )
                            epilogue(psum, ds[mi], m0 + mi, nq)

    nc.compile()
    return nc


_NC = None


def _get_nc():
    global _NC
    if _NC is None:
        _NC = _build()
    return _NC


def build_in_maps(a, w, b):
    """Host-side prep: shard A, pre-transpose/block to bf16, cast W to bf16."""
    a = np.ascontiguousarray(a, dtype=np.float32)
    w = np.ascontiguousarray(w, dtype=np.float32)
    b = np.ascontiguousarray(b, dtype=np.float32)
    # w_blk[g, kp, kk, n] = W[(g*KGS+kk)*P + kp, n], bf16
    w_blk = np.ascontiguousarray(
        np.transpose(w.astype(BF16).reshape(KG, KGS, P, N), (0, 2, 1, 3))
    )
    w0 = np.ascontiguousarray(w[:, 0].reshape(1, K))
    in_maps = []
    for i in range(N_CORES):
        a_sh = a[i * M_SHARD : (i + 1) * M_SHARD]
        # at[pr, kp, ko, mm] = a_sh[pr*256+mm, ko*128+kp]
        at = np.transpose(
            a_sh.reshape(PAIRS, MP, KO, P), (0, 3, 2, 1)
        ).astype(BF16)
        in_maps.append(
            {"inputs": a_sh, "at": at, "w": w_blk, "b": b, "w0": w0}
        )
    return in_maps


def kernel(**inputs: np.ndarray) -> np.ndarray:
    a = inputs["inputs"]
    assert a.shape == (B_FULL, K), a.shape
    nc = _get_nc()
    in_maps = build_in_maps(a, inputs["w"], inputs["b"])
    res = run_bass_kernel_spmd(nc, in_maps, core_ids=list(range(N_CORES)))
    return np.concatenate([res.results[i]["out"] for i in range(N_CORES)], axis=0)


# revision 15
# speedup vs baseline: 1.1727x; 1.0214x over previous
"""TRN2 Bass kernel for nn_LinearBinary: out = (A @ W + b) +/- 1 per-row.

    A: [8192, 2048] f32, W: [2048, 2048] f32, b: [2048] f32
    C = A @ W + b;  cond = C[:, :1] > 0.5;  out = where(cond, C+1, C-1)

Sharding: data-parallel over the 8192-row batch across 8 NeuronCores
(1024 rows/core); W and b replicated. SPMD - one program, per-core shards
via in_maps.

Per-core kernel (v3 — PE-minimal, DMA-tuned):
  - W shipped as bf16 (8MB) and kept fully resident in SBUF as 4
    ko-groups, streamed on the sync (SP) HWDGE queue with 4KB packets.
  - A^T shipped pre-transposed bf16 from the host, blocked per m-pair as
    [pair, kp, ko, m] so each partition line is one contiguous 8KB run:
    DMA arbitration is packet-round-robin, so packet size IS bandwidth
    share — 512B A^T packets get starved 8:1 by 4KB W packets.
  - PE does ZERO transposes — only the 512 bf16 matmuls (1 cyc/row,
    262144 cycles ~= 109us at 2.4GHz).
  - m-tiles in PAIRS across all 8 PSUM banks. Pair 0 runs ko-group-outer
    so its matmuls stream right behind the W group DMAs; pairs 1-3 run
    ko-inner per psum so each [128,512] tile closes early and its
    epilogue+store overlaps the remaining matmuls (kills the end-of-pair
    store burst and the final tail).
  - The row condition needs exact fp32 C[:, 0] (min |C0-0.5| margin on
    this data is ~4.4e-4; bf16 A would flip rows): computed on
    gpsimd (mult) + DVE (reduce) from the natural-layout fp32 A.
  - Epilogue fuses (psum + (-+1)) + b in one scalar_tensor_tensor per
    [128, 512] tile; stores go on the sync queue (idle once W landed).
"""

import sys

for _p in ("/opt/trn_rl_repo", "/root/.axon_site/_ro/trn_rl_repo"):
    if _p not in sys.path:
        sys.path.append(_p)

import ml_dtypes
import numpy as np

import concourse.bacc as bacc
import concourse.mybir as mybir
import concourse.tile as tile
from concourse.bass_utils import run_bass_kernel_spmd

dt = mybir.dt
Alu = mybir.AluOpType
BF16 = np.dtype(ml_dtypes.bfloat16)

P = 128
K = 2048
N = 2048
B_FULL = 8192
N_CORES = 8
M_SHARD = B_FULL // N_CORES  # 1024 rows per core
M_TILES = M_SHARD // P  # 8
KO = K // P  # 16
KG = 8  # W ko-groups (DMA granularity)
KGS = KO // KG  # 2 ko per group
NQ = 4  # PSUM n-chunks
N_SUB = N // NQ  # 512
PAIRS = M_TILES // 2  # m-tiles processed 2 at a time (8 PSUM banks)
MP = 2 * P  # rows per pair


def _build():
    nc = bacc.Bacc("TRN2", target_bir_lowering=False, debug=False, num_devices=N_CORES)

    a = nc.dram_tensor("inputs", [M_SHARD, K], dt.float32, kind="ExternalInput")
    # A^T, host-blocked per m-pair: at[pr, kp, ko, mm] = A[pr*256+mm, ko*128+kp]
    at = nc.dram_tensor("at", [PAIRS, P, KO, MP], dt.bfloat16, kind="ExternalInput")
    # W host-blocked per ko-group: w[g, kp, kk, n] = W[(g*KGS+kk)*128+kp, n]
    # so each partition line is one contiguous 8KB run (big DMA packets win
    # the packet-round-robin arbitration).
    w = nc.dram_tensor("w", [KG, P, KGS, N], dt.bfloat16, kind="ExternalInput")
    b = nc.dram_tensor("b", [N], dt.float32, kind="ExternalInput")
    # W[:, 0] pre-sliced on host: a strided 4-byte column-gather DMA is fatal
    # on HW (NRT_EXEC_UNIT_UNRECOVERABLE), so ship the 8KB row directly.
    w0 = nc.dram_tensor("w0", [1, K], dt.float32, kind="ExternalInput")
    out = nc.dram_tensor("out", [M_SHARD, N], dt.float32, kind="ExternalOutput")

    with tile.TileContext(nc) as tc:
        with (
            tc.tile_pool(name="consts", bufs=1) as consts,
            tc.tile_pool(name="wg", bufs=1) as wg_pool,
            tc.tile_pool(name="atp", bufs=1) as at_pool,
            tc.tile_pool(name="anat", bufs=2) as anat_pool,
            tc.tile_pool(name="scr", bufs=2) as scr_pool,
            tc.tile_pool(name="dsm", bufs=1) as d_pool,
            tc.tile_pool(name="outs", bufs=4) as out_pool,
            tc.tile_pool(name="psc", bufs=8, space="PSUM") as psum_pool,
        ):
            # b and W[:, 0] broadcast to all partitions (scalar queue: tiny)
            b_row = consts.tile([1, N], dt.float32, tag="b_row")
            nc.scalar.dma_start(b_row[:], b.ap().unsqueeze(0))
            b128 = consts.tile([P, N], dt.float32, tag="b128")
            nc.gpsimd.partition_broadcast(b128[:], b_row[:])
            w0_row = consts.tile([1, K], dt.float32, tag="w0_row")
            nc.sync.dma_start(w0_row[:], w0.ap())
            w0b = consts.tile([P, K], dt.float32, tag="w0b")
            nc.gpsimd.partition_broadcast(w0b[:], w0_row[:])

            # W resident, bf16, 8 ko-groups of 1MB. The bandwidth cap is
            # per-HWDGE-queue (~180GB/s each), so split W across BOTH
            # queues: sync gets g0,g2,g3,g6; scalar gets g1,g4,g5,g7
            # behind atp0 (which gates the first matmul). Pair 0 consumes
            # groups in merged arrival order CONSUME0 below.
            atps = []
            for pr in range(PAIRS):
                atp = at_pool.tile([P, KO, MP], dt.bfloat16, tag=f"atp{pr}", name=f"atp{pr}")
                atps.append(atp)

            def load_atp(pr, eng):
                eng.dma_start(atps[pr][:], at.ap()[pr])

            load_atp(0, nc.scalar)

            wgs = [None] * KG
            issue = [(0, nc.sync), (1, nc.scalar), (2, nc.sync), (4, nc.scalar),
                     (3, nc.sync), (5, nc.scalar), (6, nc.sync), (7, nc.scalar)]
            for g, eng in issue:
                wg = wg_pool.tile([P, KGS, N], dt.bfloat16, tag=f"wg{g}", name=f"wg{g}")
                eng.dma_start(wg[:], w.ap()[g])
                wgs[g] = wg
            # expected arrival: sync g0@~15,g2@~20,g3@~26,g6@~31;
            # scalar atp0@~15,g1@~20,g4@~26,g5@~31,g7@~37
            CONSUME0 = [0, 1, 2, 4, 3, 6, 5, 7]

            load_atp(1, nc.scalar)

            def cond_m(m, eng):
                # exact fp32 condition: d = (A[m-tile] @ w0 + b0 > 0.5) ? +1 : -1
                a_nat = anat_pool.tile([P, K], dt.float32, tag="a_nat", name="a_nat")
                eng.dma_start(a_nat[:], a.ap()[m * P : (m + 1) * P, :])
                scratch = scr_pool.tile([P, K], dt.float32, tag="scratch", name="scratch")
                c0 = d_pool.tile([P, 1], dt.float32, tag=f"c0_{m}", name=f"c0_{m}")
                nc.vector.tensor_tensor(scratch[:], a_nat[:], w0b[:], Alu.mult)
                nc.vector.tensor_reduce(c0[:], scratch[:], mybir.AxisListType.X, Alu.add)
                g = d_pool.tile([P, 1], dt.float32, tag=f"g_{m}", name=f"g_{m}")
                nc.vector.tensor_scalar(
                    g[:], c0[:], b128[:, 0:1], 0.5, Alu.add, Alu.is_gt
                )
                d = d_pool.tile([P, 1], dt.float32, tag=f"d_{m}", name=f"d_{m}")
                nc.vector.tensor_scalar(d[:], g[:], 2.0, -1.0, Alu.mult, Alu.add)
                return d

            def epilogue(psum, d, m, nq):
                out_sb = out_pool.tile([P, N_SUB], dt.float32, tag="out_sb", name="out_sb")
                nc.vector.scalar_tensor_tensor(
                    out_sb[:],
                    psum[:],
                    d[:],
                    b128[:, nq * N_SUB : (nq + 1) * N_SUB],
                    Alu.add,
                    Alu.add,
                )
                nc.sync.dma_start(
                    out.ap()[m * P : (m + 1) * P, nq * N_SUB : (nq + 1) * N_SUB],
                    out_sb[:],
                )

            for pr in range(PAIRS):
                m0 = 2 * pr
                # pair 0's conditions ride the sync queue (free after its W
                # half); later pairs' ride scalar behind the atp loads
                ce = nc.sync if pr == 0 else nc.scalar
                ds = [cond_m(m0, ce), cond_m(m0 + 1, ce)]
                if pr == 0:
                    load_atp(2, nc.scalar)
                    load_atp(3, nc.scalar)
                atp = atps[pr]

                if pr == 0:
                    # ko-group outer in DMA arrival order, streaming right
                    # behind the W group DMAs
                    psums = [
                        [
                            psum_pool.tile(
                                [P, N_SUB], dt.float32, name=f"ps_{mi}_{nq}", tag="psum"
                            )
                            for nq in range(NQ)
                        ]
                        for mi in range(2)
                    ]
                    for gi, g4 in enumerate(CONSUME0):
                        for kk in range(KGS):
                            ko = g4 * KGS + kk
                            for mi in range(2):
                                for nq in range(NQ):
                                    nc.tensor.matmul(
                                        psums[mi][nq][:],
                                        atp[:, ko, mi * P : (mi + 1) * P],
                                        wgs[g4][:, kk, nq * N_SUB : (nq + 1) * N_SUB],
                                        start=(gi == 0 and kk == 0),
                                        stop=(gi == KG - 1 and kk == KGS - 1),
                                    )
                    for mi in range(2):
                        for nq in range(NQ):
                            epilogue(psums[mi][nq], ds[mi], m0 + mi, nq)
                else:
                    # W fully resident: ko-inner per psum so each tile closes
                    # early and its epilogue+store overlaps remaining matmuls
                    for mi in range(2):
                        for nq in range(NQ):
                            psum = psum_pool.tile(
                                [P, N_SUB], dt.float32, name="ps", tag="psum"
                            )
                            for ko in range(KO):
                                nc.tensor.matmul(
                                    psum[:],
                                    atp[:, ko, mi * P : (mi + 1) * P],
                                    wgs[ko // KGS][:, ko % KGS, nq * N_SUB : (nq + 1) * N_SUB],
                                    start=(ko == 0),
                                    stop=(ko == KO - 1),
                                )
                            epilogue(psum, ds[mi], m0 + mi, nq)

    nc.compile()
    return nc


_NC = None


def _get_nc():
    global _NC
    if _NC is None:
        _NC = _build()
    return _NC


def build_in_maps(a, w, b):
    """Host-side prep: shard A, pre-transpose/block to bf16, cast W to bf16."""
    a = np.ascontiguousarray(a, dtype=np.float32)
    w = np.ascontiguousarray(w, dtype=np.float32)
    b = np.ascontiguousarray(b, dtype=np.float32)
    # w_blk[g, kp, kk, n] = W[(g*KGS+kk)*P + kp, n], bf16
    w_blk = np.ascontiguousarray(
        np.transpose(w.astype(BF16).reshape(KG, KGS, P, N), (0, 2, 1, 3))
    )
    w0 = np.ascontiguousarray(w[:, 0].reshape(1, K))
    in_maps = []
    for i in range(N_CORES):
        a_sh = a[i * M_SHARD : (i + 1) * M_SHARD]
        # at[pr, kp, ko, mm] = a_sh[pr*256+mm, ko*128+kp]
        at = np.transpose(
            a_sh.reshape(PAIRS, MP, KO, P), (0, 3, 2, 1)
        ).astype(BF16)
        in_maps.append(
            {"inputs": a_sh, "at": at, "w": w_blk, "b": b, "w0": w0}
        )
    return in_maps


def kernel(**inputs: np.ndarray) -> np.ndarray:
    a = inputs["inputs"]
    assert a.shape == (B_FULL, K), a.shape
    nc = _get_nc()
    in_maps = build_in_maps(a, inputs["w"], inputs["b"])
    res = run_bass_kernel_spmd(nc, in_maps, core_ids=list(range(N_CORES)))
    return np.concatenate([res.results[i]["out"] for i in range(N_CORES)], axis=0)
